# revision 65
# baseline (speedup 1.0000x reference)
"""GCN encoder (2-layer, BN, residual) on 8 Trainium2 NeuronCores.

Sharding: nodes partitioned contiguously across 8 cores (6250 each). Edges
bucketed by dst shard on host (integer-only preprocessing). All float math
runs on device.

SpMM: per dst tile, edges split into three source buckets:
  M — src in this core's own shard: gathered from the LOCAL x16/h16 shard
      buffer, so these dma_gather calls run concurrently with the table
      AllGather (which they don't depend on).
  A — src in rows [0, 32768) of the AllGathered table.
  B — src in rows [N-32768, N). A/B membership for srcs in the overlap is
      balanced per tile to equalize chunk counts.
Gathered 128-edge chunks (rows->partitions) are scatter-added into
m^T[feat, dst] PSUM tiles via one-hot selector matmuls on PE (selector from
iota==slot compare on DVE). Gather descriptor generation on the Q7 cores is
the kernel bottleneck (~2.5 ns/idx across the 4 SWDGE queues), so calls are
large (<=2048 idx), buffered deep (6 pairs in flight), and bucket index
lists are sorted by src for HBM locality. Because gathers from the Shared
(collective-output) address space run ~1.6x slower, each layer copies the
AllGathered table to plain DRAM on both HWDGE engines; the first
N_SLOW_PAIRS pairs gather straight from the Shared table so the Q7 queues
stay busy while the copy drains.

BN stats: per-core partial sums + 1KB AllReduce.
"""

import sys

sys.path.insert(0, "/opt/trn_rl_repo")

import numpy as np

P = 128
N_CORES = 8
EPS = 1e-5

_F16 = "float16"

MAX_CALL_IDX = 2048  # max indices per dma_gather call
N_SLOW_PAIRS = 4  # pairs whose A/B gathers read the Shared AG output directly
DMA_SCRATCH = 32768
N_SWDGE_QUEUES = 4
CM = 2  # chunks per tile reserved for own-shard (M) sources; overflow -> A/B


def _cdiv(a, b):
    return -(-a // b)


# ---------------------------------------------------------------------------
# host-side integer preprocessing (indices only; no float arithmetic on data)
# ---------------------------------------------------------------------------


def _wrap_idx_image(idx_list):
    """int16 index list (len % 16 == 0) -> [128, len/16] SBUF image."""
    n = idx_list.shape[0]
    assert n % 16 == 0
    img16 = idx_list.reshape(n // 16, 16).T  # [16, n/16]
    return np.tile(img16, (8, 1)).astype(np.int16)  # [128, n/16]


def _host_prep(src, dst, n_nodes):
    NC = N_CORES
    SH = n_nodes // NC
    assert SH * NC == n_nodes
    T = _cdiv(SH, P)
    A_LIM = 32768
    T_FULL = _cdiv(n_nodes, P)  # full-table tiles (padded)
    NN_PAD = T_FULL * P
    B_OFF = NN_PAD - 32768  # tile-aligned so the B view is its own tensor
    assert B_OFF % P == 0 and 0 <= B_OFF < A_LIM

    src = np.asarray(src, np.int64)
    dst = np.asarray(dst, np.int64)

    # pass 1: bucket per (core, tile) into M / A / B index+slot lists
    per_core = []
    CA = CB = 1
    for k in range(NC):
        m = (dst >= k * SH) & (dst < (k + 1) * SH)
        s = src[m]
        dl = dst[m] - k * SH
        t_idx = dl // P
        slot = dl % P
        tiles = []
        for t in range(T):
            tm = t_idx == t
            ss = s[tm]
            sl = slot[tm]
            mine = (ss >= k * SH) & (ss < (k + 1) * SH)
            mi = np.nonzero(mine)[0]
            oi = np.nonzero(~mine)[0]
            take = mi[: CM * P]
            rest = np.concatenate([mi[CM * P :], oi])
            M_idx = (ss[take] - k * SH).astype(np.int64)
            M_slot = sl[take]
            # A/B assignment with per-tile balancing
            rs = ss[rest]
            rsl = sl[rest]
            a_forced = rs < B_OFF
            b_forced = rs >= A_LIM
            free = ~a_forced & ~b_forced
            na, nb = int(a_forced.sum()), int(b_forced.sum())
            nf = int(free.sum())
            # assign free (overlap-region) edges to balance A/B counts
            a_extra = max(0, min(nf, ((na + nb + nf) // 2) - na))
            fidx = np.nonzero(free)[0]
            to_a = np.zeros(rest.shape[0], bool)
            to_a[a_forced] = True
            to_a[fidx[:a_extra]] = True
            A_idx = rs[to_a]
            A_slot = rsl[to_a]
            B_idx = rs[~to_a] - B_OFF
            B_slot = rsl[~to_a]

            def _srt(iv, sl):
                o = np.argsort(iv, kind="stable")
                return iv[o], sl[o]

            M_idx, M_slot = _srt(M_idx, M_slot)
            A_idx, A_slot = _srt(A_idx, A_slot)
            B_idx, B_slot = _srt(B_idx, B_slot)
            tiles.append((M_idx, M_slot, A_idx, A_slot, B_idx, B_slot))
            CA = max(CA, _cdiv(len(A_idx), P))
            CB = max(CB, _cdiv(len(B_idx), P))
        per_core.append(tiles)

    NCHT = CM + CA + CB
    n_chunks = T * NCHT
    pairs = [(2 * b, min(2 * b + 1, T - 1)) for b in range(_cdiv(T, 2))]

    # pass 2: pack per-core images with uniform capacities
    cores = []
    for k in range(NC):
        tiles = per_core[k]
        # per-tile chunk-major index buffers (pad idx 0, slot 255)
        bufM = np.zeros((T, CM * P), np.int16)
        bufA = np.zeros((T, CA * P), np.int16)
        bufB = np.zeros((T, CB * P), np.int16)
        slots = np.full((n_chunks, P), 255.0, np.float16)
        for t in range(T):
            M_idx, M_slot, A_idx, A_slot, B_idx, B_slot = tiles[t]
            for ci, (Cc, buf, iv, sl) in enumerate(
                ((CM, bufM, M_idx, M_slot), (CA, bufA, A_idx, A_slot),
                 (CB, bufB, B_idx, B_slot))
            ):
                n = len(iv)
                assert n <= Cc * P
                buf[t, :n] = iv.astype(np.int16)
                base = t * NCHT + (0 if ci == 0 else (CM if ci == 1 else CM + CA))
                for c in range(Cc):
                    lo, hi = c * P, min((c + 1) * P, n)
                    if hi > lo:
                        slots[base + c, : hi - lo] = sl[lo:hi].astype(np.float16)

        # gather-call index images; per pair: M call, A calls, B calls
        imgs = []
        offs = {"M": [], "A": [], "B": []}
        col = 0
        for t0, t1 in pairs:
            tl = [t0] if t0 == t1 else [t0, t1]
            for key, Cc, buf in (("M", CM, bufM), ("A", CA, bufA), ("B", CB, bufB)):
                lst = np.concatenate([buf[t] for t in tl])
                # split into calls of <= MAX_CALL_IDX
                call_list = []
                p0 = 0
                while p0 < lst.shape[0]:
                    p1 = min(p0 + MAX_CALL_IDX, lst.shape[0])
                    img = _wrap_idx_image(lst[p0:p1])
                    call_list.append((col, img.shape[1], p1 - p0, p0 // P))
                    col += img.shape[1]
                    imgs.append(img)
                    p0 = p1
                offs[key].append(call_list)
        idx_img = np.concatenate(imgs, axis=1)  # [128, col]

        outdeg = np.bincount(src, minlength=n_nodes).astype(np.int64)
        indeg = np.bincount(dst, minlength=n_nodes).astype(np.int64)
        mine = slice(k * SH, (k + 1) * SH)

        def _cols(d):
            v = np.ones(T * P, np.float32)
            v[:SH] = d[mine].astype(np.float32)
            return v.reshape(T, P).T.copy()  # [P, T]

        dof = np.ones(NN_PAD, np.float32)
        dof[:n_nodes] = outdeg.astype(np.float32)

        cores.append(
            dict(
                idx_img=idx_img,
                slotT=slots.T.copy(),  # [P, n_chunks] fp16
                deg_out=_cols(outdeg),
                deg_in=_cols(indeg),
                deg_out_full=dof.reshape(T_FULL, P).T.copy(),  # [P, T_FULL]
                offs=offs,
            )
        )

    meta = dict(
        SH=SH,
        T=T,
        T_FULL=T_FULL,
        NN_PAD=NN_PAD,
        A_LIM=A_LIM,
        B_OFF=B_OFF,
        CA=CA,
        CB=CB,
        NCHT=NCHT,
        n_chunks=n_chunks,
        pairs=pairs,
        idx_cols=cores[0]["idx_img"].shape[1],
        n_nodes=n_nodes,
        offs=cores[0]["offs"],
    )
    for c in cores[1:]:
        assert c["offs"] == meta["offs"]
        assert c["idx_img"].shape == cores[0]["idx_img"].shape
    return meta, cores


# ---------------------------------------------------------------------------
# device program (identical on all cores; all data-dependence through SBUF)
# ---------------------------------------------------------------------------


def _build_program(meta):
    import concourse.bacc as bacc
    import concourse.tile as tile
    from concourse import mybir
    from concourse.masks import make_identity

    f32 = mybir.dt.float32
    f16 = getattr(mybir.dt, _F16)
    Alu = mybir.AluOpType
    Act = mybir.ActivationFunctionType

    SH, T = meta["SH"], meta["T"]
    A_LIM, B_OFF = meta["A_LIM"], meta["B_OFF"]
    CA, CB, NCHT = meta["CA"], meta["CB"], meta["NCHT"]
    NN = meta["n_nodes"]
    pairs = meta["pairs"]

    nc = bacc.Bacc(
        "TRN2",
        target_bir_lowering=False,
        debug=False,
        num_devices=N_CORES,
        dynamic_dma_scratch_size=DMA_SCRATCH,
        num_swdge_queues=N_SWDGE_QUEUES,
    )

    # ---- I/O -------------------------------------------------------------
    SHP = T * P
    x_shard = nc.dram_tensor("x_shard", [SHP, P], f32, kind="ExternalInput")
    W1_t = nc.dram_tensor("W1", [P, P], f32, kind="ExternalInput")
    W2_t = nc.dram_tensor("W2", [P, P], f32, kind="ExternalInput")
    gm1 = nc.dram_tensor("gamma1", [P, 1], f32, kind="ExternalInput")
    bt1 = nc.dram_tensor("beta1", [P, 1], f32, kind="ExternalInput")
    gm2 = nc.dram_tensor("gamma2", [P, 1], f32, kind="ExternalInput")
    bt2 = nc.dram_tensor("beta2", [P, 1], f32, kind="ExternalInput")
    iota_t = nc.dram_tensor("iota", [P, P], f16, kind="ExternalInput")
    idx_t = nc.dram_tensor("idx_img", [P, meta["idx_cols"]], mybir.dt.int16,
                           kind="ExternalInput")
    slot_t = nc.dram_tensor("slotT", [P, meta["n_chunks"]], f16,
                            kind="ExternalInput")
    dego_t = nc.dram_tensor("deg_out", [P, T], f32, kind="ExternalInput")
    degi_t = nc.dram_tensor("deg_in", [P, T], f32, kind="ExternalInput")
    T_FULL, NN_PAD = meta["T_FULL"], meta["NN_PAD"]
    out_t = nc.dram_tensor("out", [SHP, P], f32, kind="ExternalOutput")

    GROUPS = [(g, min(g + 8, T)) for g in range(0, T, 8)]
    TILE_A = A_LIM // P  # full-table tiles [0, TILE_A) belong to the A view
    TILE_B0 = B_OFF // P  # tiles [TILE_B0, T_FULL) belong to the B view

    with tile.TileContext(nc) as tc:
        with (
            tc.tile_pool(name="cst", bufs=1) as cst,
            tc.tile_pool(name="big", bufs=1) as big,
            tc.tile_pool(name="gm", bufs=1) as gmp,
            tc.tile_pool(name="gat", bufs=6) as gat,
            tc.tile_pool(name="wrk", bufs=3) as wrk,
            tc.tile_pool(name="ps", bufs=2, space="PSUM") as ps,
            tc.tile_pool(name="dram", bufs=1, space="DRAM") as dram,
        ):
            # ---- degree normalizers -------------------------------------
            d_out = cst.tile([P, T], f32)
            d_in = cst.tile([P, T], f32)
            for deg_dram, d_sb in ((dego_t, d_out), (degi_t, d_in)):
                raw = wrk.tile([P, T], f32, tag="degraw")
                nc.sync.dma_start(raw[:], deg_dram[:])
                nc.vector.tensor_scalar_max(raw[:], raw[:], 1.0)
                nc.scalar.sqrt(raw[:], raw[:])
                nc.vector.reciprocal(d_sb[:], raw[:])

            # ---- layer-1 gather tables, built locally from the full x ----
            # (kills the first AllGather and the Shared->plain copy). Two
            # separate plain tensors so A gathers start before B is built.
            x16_shard = dram.tile([SHP, P], f16)
            x_re = x_shard.rearrange("(t p) f -> p t f", p=P)
            x16_re = x16_shard.rearrange("(t p) f -> p t f", p=P)
            for gi, (g0, g1) in enumerate(GROUPS):
                gs = g1 - g0
                eng = nc.sync if gi % 2 == 0 else nc.scalar
                xg = wrk.tile([P, gs, P], f32, tag="xg", bufs=2, name=f"xg{g0}")
                eng.dma_start(xg[:], x_re[:, g0:g1, :])
                x16g = wrk.tile([P, gs, P], f16, tag="x16g", bufs=2,
                                name=f"x16g{g0}")
                nc.vector.tensor_tensor(
                    out=x16g[:],
                    in0=xg[:],
                    in1=d_out[:, g0:g1, None].to_broadcast([P, gs, P]),
                    op=Alu.mult,
                )
                eng.dma_start(x16_re[:, g0:g1, :], x16g[:])

            x16_plain = dram.tile([NN, P], f16, name="x16_plain")
            h16_plain = dram.tile([NN, P], f16, name="h16_plain")
            x16_full = dram.tile([NN, P], f16, addr_space="Shared")
            nc.gpsimd.collective_compute(
                "AllGather",
                Alu.bypass,
                replica_groups=[list(range(N_CORES))],
                ins=[x16_shard[0:SH, :].opt()],
                outs=[x16_full.opt()],
            )

            def table_copy(shared_tbl, plain_tbl):
                NCHK = 8
                for i, c0 in enumerate(range(0, NN, NN // NCHK)):
                    c1 = min(c0 + NN // NCHK, NN)
                    eng = nc.sync if i % 2 == 0 else nc.scalar
                    eng.dma_start(plain_tbl[c0:c1, :], shared_tbl[c0:c1, :])

            table_copy(x16_full, x16_plain)

            # ---- constants / static data --------------------------------
            ident = cst.tile([P, P], f32)
            make_identity(nc, ident[:])
            W1s = cst.tile([P, P], f32)
            W2s = cst.tile([P, P], f32)
            iota = cst.tile([P, P], f16)
            nc.sync.dma_start(W1s[:], W1_t[:])
            nc.sync.dma_start(W2s[:], W2_t[:])
            nc.sync.dma_start(iota[:], iota_t[:])
            idx_sb = cst.tile([P, meta["idx_cols"]], mybir.dt.int16)
            nc.sync.dma_start(idx_sb[:], idx_t[:])
            slot_sb = cst.tile([P, meta["n_chunks"]], f16)
            nc.sync.dma_start(slot_sb[:], slot_t[:])
            gm1s = cst.tile([P, 1], f32)
            bt1s = cst.tile([P, 1], f32)
            gm2s = cst.tile([P, 1], f32)
            bt2s = cst.tile([P, 1], f32)
            nc.sync.dma_start(gm1s[:], gm1[:])
            nc.sync.dma_start(bt1s[:], bt1[:])
            nc.sync.dma_start(gm2s[:], gm2[:])
            nc.sync.dma_start(bt2s[:], bt2[:])
            W1h = cst.tile([P, P], f16)
            W2h = cst.tile([P, P], f16)
            nc.vector.tensor_copy(W1h[:], W1s[:])
            nc.vector.tensor_copy(W2h[:], W2s[:])
            ident16 = cst.tile([P, P], f16)
            nc.vector.tensor_copy(ident16[:], ident[:])

            # d_in broadcast rows: din_bc[:, t*P+j] = d_in[j, t] for all rows
            din_bc = big.tile([P, T * P], f16)
            for t in range(T):
                bc_ps = ps.tile([P, P], f32, tag="tp")
                nc.tensor.transpose(
                    out=bc_ps[:],
                    in_=d_in[:, t : t + 1].to_broadcast([P, P]),
                    identity=ident[:],
                )
                nc.vector.tensor_copy(din_bc[:, t * P : (t + 1) * P], bc_ps[:])

            # persistent stores
            hpre = big.tile([P, T * P], f32)   # pre-BN activations [feat, dst]
            h1 = big.tile([P, T * P], f32)     # post-BN/relu layer-1 output
            h16_shard = dram.tile([SHP, P], f16)
            h16_full = dram.tile([NN, P], f16, addr_space="Shared")

            gq = [0]

            def gather_calls(pool_tile, call_list, view, tag):
                """Issue the dma_gather calls for one (pair, bucket)."""
                for col, wcols, nidx, c0 in call_list:
                    nch = nidx // P
                    nc.gpsimd.dma_gather(
                        pool_tile[:, c0 : c0 + nch, :],
                        view,
                        idx_sb[:, col : col + wcols],
                        nidx,
                        nidx,
                        P,
                        single_packet=False,
                        queue_num=gq[0] % N_SWDGE_QUEUES,
                    )
                    gq[0] += 1


            def gconv_layer(pA, pB, sA, sB, tableM, W_sb, s1_cols,
                            s2_cols, lt, n_slow, after_m=None):
                # phase 0: all M gathers (independent of the AllGather)
                gMs = []
                for ip, (t0, t1) in enumerate(pairs):
                    g = gmp.tile([P, 2 * CM, P], f16, tag="gM", bufs=len(pairs),
                                 name=f"gM{lt}_{ip}")
                    gather_calls(g, meta["offs"]["M"][ip], tableM, "M")
                    gMs.append(g)
                if after_m is not None:
                    after_m()

                # main loop
                for ip, (t0, t1) in enumerate(pairs):
                    tableA, tableB = (sA, sB) if ip < n_slow else (pA, pB)
                    tl = [t0] if t0 == t1 else [t0, t1]
                    gA = gat.tile([P, 2 * CA, P], f16, tag="gA")
                    gather_calls(gA, meta["offs"]["A"][ip], tableA, "A")
                    gB = gat.tile([P, 2 * CB, P], f16, tag="gB")
                    gather_calls(gB, meta["offs"]["B"][ip], tableB, "B")
                    gM = gMs[ip]
                    sels = []
                    mTs_ps = []
                    for ti, t in enumerate(tl):
                        cid0 = t * NCHT
                        sel = wrk.tile([P, NCHT, P], f16, tag="sel", bufs=4,
                                       name=f"sel{ti}")
                        nc.vector.tensor_tensor(
                            out=sel[:],
                            in0=slot_sb[:, cid0 : cid0 + NCHT][:, :, None]
                            .to_broadcast([P, NCHT, P]),
                            in1=iota[:, None, :].to_broadcast([P, NCHT, P]),
                            op=Alu.is_equal,
                        )
                        sels.append(sel)
                        mTs_ps.append(ps.tile([P, P], f32, tag="mT", bufs=4,
                                              name=f"mT{ti}"))
                    # interleave the two tiles' accumulation chains
                    for c in range(NCHT):
                        if c < CM:
                            buf, Cc, cc = gM, CM, c
                        elif c < CM + CA:
                            buf, Cc, cc = gA, CA, c - CM
                        else:
                            buf, Cc, cc = gB, CB, c - CM - CA
                        for ti in range(len(tl)):
                            nc.tensor.matmul(
                                out=mTs_ps[ti][:],
                                lhsT=buf[:, ti * Cc + cc, :],
                                rhs=sels[ti][:, c, :],
                                start=(c == 0),
                                stop=(c == NCHT - 1),
                            )
                    for ti, t in enumerate(tl):
                        mTs = wrk.tile([P, P], f16, tag="mTs")
                        nc.vector.tensor_tensor(
                            out=mTs[:],
                            in0=mTs_ps[ti][:],
                            in1=din_bc[:, t * P : (t + 1) * P],
                            op=Alu.mult,
                        )
                        hp = ps.tile([P, P], f32, tag="hp")
                        nc.tensor.matmul(
                            out=hp[:], lhsT=W_sb[:], rhs=mTs[:], start=True,
                            stop=True,
                        )
                        nc.vector.tensor_scalar(
                            hpre[:, t * P : (t + 1) * P],
                            hp[:],
                            1.0,
                            None,
                            Alu.mult,
                            Alu.add,
                            accum_out=s1_cols[:, t : t + 1],
                        )
                        sq = wrk.tile([P, P], f16, tag="sq")
                        nc.scalar.activation(
                            sq[:],
                            hpre[:, t * P : (t + 1) * P],
                            Act.Square,
                            accum_out=s2_cols[:, t : t + 1],
                        )

            def bn_coeffs(s1_cols, s2_cols, gam, bet, tag):
                stats_in = dram.tile([P, 2], f32, name=f"stats_in_{tag}")
                stats_out = dram.tile(
                    [P, 2], f32, addr_space="Shared", name=f"stats_out_{tag}"
                )
                pack = wrk.tile([P, 2], f32, tag="pack")
                nc.vector.tensor_reduce(
                    pack[:, 0:1], s1_cols[:], axis=mybir.AxisListType.X, op=Alu.add
                )
                nc.vector.tensor_reduce(
                    pack[:, 1:2], s2_cols[:], axis=mybir.AxisListType.X, op=Alu.add
                )
                nc.sync.dma_start(stats_in[:], pack[:])
                nc.gpsimd.collective_compute(
                    "AllReduce",
                    Alu.add,
                    replica_groups=[list(range(N_CORES))],
                    ins=[stats_in.opt()],
                    outs=[stats_out.opt()],
                )
                glob = wrk.tile([P, 2], f32, tag="glob")
                nc.sync.dma_start(glob[:], stats_out[:])
                mo = wrk.tile([P, 4], f32, tag="mo")
                nc.vector.tensor_scalar(mo[:, 0:2], glob[:], 1.0 / NN, None, Alu.mult)
                nc.vector.tensor_tensor(
                    out=mo[:, 3:4], in0=mo[:, 0:1], in1=mo[:, 0:1], op=Alu.mult
                )
                nc.vector.tensor_tensor(
                    out=mo[:, 2:3], in0=mo[:, 1:2], in1=mo[:, 3:4], op=Alu.subtract
                )
                nc.vector.tensor_scalar_add(mo[:, 2:3], mo[:, 2:3], EPS)
                nc.scalar.sqrt(mo[:, 2:3], mo[:, 2:3])
                a_c = cst.tile([P, 2], f32, name=f"a_c_{gam.name}")
                nc.vector.reciprocal(a_c[:, 0:1], mo[:, 2:3])
                nc.vector.tensor_tensor(
                    out=a_c[:, 0:1], in0=a_c[:, 0:1], in1=gam[:], op=Alu.mult
                )
                nc.vector.tensor_tensor(
                    out=a_c[:, 1:2], in0=a_c[:, 0:1], in1=mo[:, 0:1], op=Alu.mult
                )
                nc.vector.tensor_tensor(
                    out=a_c[:, 1:2], in0=bet[:], in1=a_c[:, 1:2], op=Alu.subtract
                )
                return a_c

            # ================= layer 1 =================
            s1a = cst.tile([P, T], f32)
            s2a = cst.tile([P, T], f32)
            gconv_layer(x16_plain[0:A_LIM, :], x16_plain[B_OFF:NN, :],
                        x16_full[0:A_LIM, :], x16_full[B_OFF:NN, :],
                        x16_shard[0:SHP, :], W1h, s1a, s2a, "l1",
                        N_SLOW_PAIRS)
            ac1 = bn_coeffs(s1a, s2a, gm1s, bt1s, "l1")

            # BN + relu -> h1 per 8-tile group so transposes/stores pipeline
            # behind the activation instead of waiting for the whole tensor
            h16_re = h16_shard.rearrange("(t p) f -> p t f", p=P)
            for gi, (g0, g1) in enumerate(GROUPS):
                gs = g1 - g0
                nc.scalar.activation(
                    h1[:, g0 * P : g1 * P], hpre[:, g0 * P : g1 * P],
                    Act.Relu, bias=ac1[:, 1:2], scale=ac1[:, 0:1],
                )
                stg = wrk.tile([P, gs, P], f16, tag="x16g", bufs=2,
                               name=f"stg{g0}")
                for t in range(g0, g1):
                    tp = ps.tile([P, P], f32, tag="tp")
                    nc.tensor.transpose(
                        out=tp[:],
                        in_=h1[:, t * P : (t + 1) * P],
                        identity=ident[:],
                    )
                    nc.vector.tensor_scalar(
                        stg[:, t - g0, :], tp[:], d_out[:, t : t + 1], None,
                        Alu.mult,
                    )
                eng = nc.sync if gi % 2 == 0 else nc.scalar
                eng.dma_start(h16_re[:, g0:g1, :], stg[:])
            nc.gpsimd.collective_compute(
                "AllGather",
                Alu.bypass,
                replica_groups=[list(range(N_CORES))],
                ins=[h16_shard[0:SH, :].opt()],
                outs=[h16_full.opt()],
            )

            # ================= layer 2 =================
            s1b = cst.tile([P, T], f32)
            s2b = cst.tile([P, T], f32)
            table_copy(h16_full, h16_plain)
            gconv_layer(h16_plain[0:A_LIM, :], h16_plain[B_OFF:NN, :],
                        h16_full[0:A_LIM, :], h16_full[B_OFF:NN, :],
                        h16_shard[0:SHP, :], W2h, s1b, s2b, "l2", N_SLOW_PAIRS)
            ac2 = bn_coeffs(s1b, s2b, gm2s, bt2s, "l2")

            # h2 = ac2*hpre + c2; out = relu(h2 + h1): transpose then relu on
            # evacuation (relu commutes with transpose); per-group pipeline
            out_re = out_t.rearrange("(t p) f -> p t f", p=P)
            for gi, (g0, g1) in enumerate(GROUPS):
                gs = g1 - g0
                nc.scalar.activation(
                    hpre[:, g0 * P : g1 * P], hpre[:, g0 * P : g1 * P],
                    Act.Identity, bias=ac2[:, 1:2], scale=ac2[:, 0:1],
                )
                nc.vector.tensor_tensor(
                    out=hpre[:, g0 * P : g1 * P], in0=hpre[:, g0 * P : g1 * P],
                    in1=h1[:, g0 * P : g1 * P], op=Alu.add,
                )
                og = wrk.tile([P, gs, P], f32, tag="xg", bufs=2, name=f"og{g0}")
                for t in range(g0, g1):
                    tp = ps.tile([P, P], f32, tag="tp")
                    nc.tensor.transpose(
                        out=tp[:],
                        in_=hpre[:, t * P : (t + 1) * P],
                        identity=ident[:],
                    )
                    nc.vector.tensor_scalar(
                        og[:, t - g0, :], tp[:], 0.0, None, Alu.max
                    )
                eng = nc.sync if gi % 2 == 0 else nc.scalar
                eng.dma_start(out_re[:, g0:g1, :], og[:])

    nc.compile()
    return nc


# ---------------------------------------------------------------------------


_CACHE = {}


def _get_program(meta):
    key = (meta["SH"], meta["T"], meta["CA"], meta["CB"], meta["idx_cols"])
    if key not in _CACHE:
        _CACHE[key] = _build_program(meta)
    return _CACHE[key]


def _build_in_maps(meta, cores, inputs):
    x = np.asarray(inputs["x"], np.float32)
    SH, T = meta["SH"], meta["T"]
    SHP = T * P
    iota = np.tile(np.arange(P, dtype=np.float16), (P, 1))
    in_maps = []
    for k in range(N_CORES):
        c = cores[k]
        xs = np.zeros((SHP, P), np.float32)
        xs[:SH] = x[k * SH : (k + 1) * SH]
        in_maps.append(
            {
                "x_shard": xs,
                "W1": np.asarray(inputs["W1"], np.float32),
                "W2": np.asarray(inputs["W2"], np.float32),
                "gamma1": np.asarray(inputs["gamma1"], np.float32).reshape(P, 1),
                "beta1": np.asarray(inputs["beta1"], np.float32).reshape(P, 1),
                "gamma2": np.asarray(inputs["gamma2"], np.float32).reshape(P, 1),
                "beta2": np.asarray(inputs["beta2"], np.float32).reshape(P, 1),
                "iota": iota,
                "idx_img": c["idx_img"],
                "slotT": c["slotT"],
                "deg_out": c["deg_out"],
                "deg_in": c["deg_in"],
            }
        )
    return in_maps


def kernel(**inputs):
    x = np.asarray(inputs["x"], np.float32)
    src = np.asarray(inputs["src"])
    dst = np.asarray(inputs["dst"])
    n_nodes = x.shape[0]

    meta, cores = _host_prep(src, dst, n_nodes)
    nc = _get_program(meta)
    in_maps = _build_in_maps(meta, cores, inputs)

    from concourse.bass_utils import run_bass_kernel_spmd

    res = run_bass_kernel_spmd(nc, in_maps, core_ids=list(range(N_CORES)))
    SH = meta["SH"]
    out = np.concatenate(
        [res.results[k]["out"][:SH] for k in range(N_CORES)], axis=0
    )
    return out.astype(np.float32)
